# revision 1
# baseline (speedup 1.0000x reference)
"""GCN (3x GCNConv + 1x1 conv) on 8 Trainium2 NeuronCores.

Strategy: node-parallel sharding (12544 padded nodes/core). Symmetric GCN
normalization folds into per-edge weights w_e = dinv[src]*dinv[dst], baked by
the host into sparse "aggregation matrices" M ([128 edge-slots, 128 dst-node]
tiles). On-device aggregation = M.T @ G on the TensorEngine (G = src rows
gathered via dma_gather), accumulated in PSUM; self-loop term is a fused
scale-add from a contiguous window load. Layer 1 aggregates raw x first
(aggregate-then-transform; x replicated to all cores), layers 2/3 are
transform-first with on-device AllGathers of h2/h3. Final 512x512 layer
consumes feature-major x1T/x2T/x3T; output returned feature-major per core and
transposed/assembled on the host.
"""

import math
import sys

import numpy as np

if "/opt/trn_rl_repo" not in sys.path:
    sys.path.insert(0, "/opt/trn_rl_repo")

import concourse.bacc as bacc
import concourse.bass as bass
import concourse.mybir as mybir
import concourse.tile as tile
from concourse.masks import make_identity

P = 128          # partition width / window size
NCORES = 8
F0, F1, F2, F3, FO = 512, 256, 128, 128, 512

# batch tile budgets (slots of 128)
B1_TILES = 8     # L1 gather batch budget ([128, 10, 512] f32 = 2.6MB)
B23_TILES = 16   # L2/3 gather batch budget ([128, 18, 128] f32 = 1.2MB)


# ---------------------------------------------------------------- host prep

def _pack_batches(tiles_per_window, budget):
    """Greedy pack windows into batches with sum(tiles) <= budget."""
    batches = []
    cur, cur_t = [], 0
    for w, t in enumerate(tiles_per_window):
        if cur and cur_t + t > budget:
            batches.append(cur)
            cur, cur_t = [], 0
        cur.append(w)
        cur_t += t
    if cur:
        batches.append(cur)
    return batches


class Sched:
    """Static (core-independent) slot schedule for one gather stream.

    Layout: batches of windows; within a batch, tiles are laid chunk-major:
    for ch in chunks: for w in batch: T[w,ch] tiles. Every (w,ch) run is a
    whole number of 128-slot tiles, so tiles never straddle windows.
    """

    def __init__(self, T_wc, budget):
        # T_wc: [n_windows, n_chunks] tile counts
        self.T_wc = T_wc
        self.n_windows, self.n_chunks = T_wc.shape
        self.batches = _pack_batches(T_wc.sum(axis=1), budget)
        self.batch_info = []   # per batch dicts
        # global tile base of (w, ch)
        self.tile_base = np.zeros((self.n_windows, self.n_chunks), np.int64)
        gt = 0
        for bw in self.batches:
            info = {"windows": bw, "slot_base": gt * P, "calls": [],
                    "win_tiles": {w: [] for w in bw}}
            bt = 0  # batch-local tile idx
            for ch in range(self.n_chunks):
                call_tiles = 0
                call_off = bt
                for w in bw:
                    t = int(T_wc[w, ch])
                    self.tile_base[w, ch] = gt + bt
                    for k in range(t):
                        info["win_tiles"][w].append(bt + k)
                    bt += t
                    call_tiles += t
                if call_tiles > 0:
                    info["calls"].append((ch, call_off, call_tiles))
            info["n_tiles"] = bt
            self.batch_info.append(info)
            gt += bt
        self.total_tiles = gt
        self.total_slots = gt * P


def _group_ranks(keys):
    """For sorted keys, rank of each element within its group."""
    n = len(keys)
    if n == 0:
        return np.zeros(0, np.int64)
    starts = np.r_[0, np.flatnonzero(np.diff(keys)) + 1]
    lens = np.diff(np.r_[starts, n])
    return np.arange(n) - np.repeat(starts, lens)


def _fill_stream(sched, w_e, ch_e, col_e, wt_e, rel_e):
    """Given a core's edges (window, chunk, col, weight, rel table idx),
    produce idx_flat [total_slots] int16 and M [128, total_slots] f32."""
    S = sched.total_slots
    idx_flat = np.zeros(S, np.int16)
    M = np.zeros((P, S), np.float32)
    if len(w_e):
        key = w_e.astype(np.int64) * sched.n_chunks + ch_e
        order = np.argsort(key, kind="stable")
        ks = key[order]
        ranks = _group_ranks(ks)
        tb = sched.tile_base[w_e[order], ch_e[order]]
        slot = (tb + ranks // P) * P + ranks % P
        idx_flat[slot] = rel_e[order].astype(np.int16)
        M[slot % P, (slot // P) * P + col_e[order]] = wt_e[order]
    return idx_flat, M


def _wrap16(idx_flat):
    """[S] -> [128, S//16] int16 (wrapped in 16 partitions, replicated x8)."""
    S = len(idx_flat)
    return np.tile(idx_flat.reshape(S // 16, 16).T, (8, 1)).copy()


def host_prep(x, W1, b1, W2, b2, W3, b3, W4, b4, edge_index, npc_real):
    N = x.shape[0]
    ncores = NCORES
    npc = ((npc_real + P - 1) // P) * P      # padded nodes per core
    n_win = npc // P
    n_pad = npc * ncores
    ch23 = n_pad // 4                        # L2/3 chunk size
    assert ch23 <= 32768 and n_pad % 4 == 0

    src = np.asarray(edge_index[0], np.int64)
    dst = np.asarray(edge_index[1], np.int64)
    deg = np.bincount(dst, minlength=N).astype(np.float64) + 1.0
    dinv = (1.0 / np.sqrt(deg)).astype(np.float32)

    def pad_id(v):
        return (v // npc_real) * npc + (v % npc_real)

    src_p = pad_id(src)
    dst_p = pad_id(dst)
    w_edge = (dinv[src] * dinv[dst]).astype(np.float32)

    x_pad = np.zeros((n_pad, F0), np.float32)
    for c in range(ncores):
        x_pad[c * npc:c * npc + npc_real] = x[c * npc_real:(c + 1) * npc_real]

    # per-core edge partitions
    core_of = dst // npc_real
    per_core = []
    for c in range(ncores):
        m = core_of == c
        per_core.append({
            "src_p": src_p[m],
            "dstrel": dst_p[m] - c * npc,
            "w": w_edge[m],
        })

    # ---- L1 compact tables
    uniq_list, inv_list = [], []
    for c in range(ncores):
        u, inv = np.unique(per_core[c]["src_p"], return_inverse=True)
        uniq_list.append(u)
        inv_list.append(inv)
    U_MAX = max(1, max(len(u) for u in uniq_list))
    assert U_MAX <= 32768, f"compact table too big: {U_MAX}"

    # ---- schedules (global max over cores)
    cnt1 = np.zeros((ncores, n_win), np.int64)
    cnt23 = np.zeros((ncores, n_win, 4), np.int64)
    for c in range(ncores):
        w_e = per_core[c]["dstrel"] // P
        ch_e = per_core[c]["src_p"] // ch23
        np.add.at(cnt1, (c, w_e), 1)
        np.add.at(cnt23, (c, w_e, ch_e), 1)
    T1 = np.ceil(cnt1.max(axis=0) / P).astype(np.int64)[:, None]   # [n_win,1]
    T23 = np.ceil(cnt23.max(axis=0) / P).astype(np.int64)         # [n_win,4]
    s1 = Sched(T1, B1_TILES)
    s23 = Sched(T23, B23_TILES)

    # ---- per-core arrays
    cores = []
    for c in range(ncores):
        pc = per_core[c]
        w_e = (pc["dstrel"] // P).astype(np.int64)
        col_e = (pc["dstrel"] % P).astype(np.int64)

        idx1, M1 = _fill_stream(
            s1, w_e, np.zeros_like(w_e), col_e, pc["w"], inv_list[c])
        ch_e = (pc["src_p"] // ch23).astype(np.int64)
        rel_e = (pc["src_p"] % ch23).astype(np.int64)
        idx23, M23 = _fill_stream(s23, w_e, ch_e, col_e, pc["w"], rel_e)

        tab = np.zeros((U_MAX, F0), np.float32)
        tab[:len(uniq_list[c])] = x_pad[uniq_list[c]]

        wself = np.zeros(npc, np.float32)
        wself[:npc_real] = dinv[c * npc_real:(c + 1) * npc_real] ** 2
        cores.append({
            "x_tab": tab,
            "x_own": x_pad[c * npc:(c + 1) * npc],
            "idx1": _wrap16(idx1),
            "M1": M1,
            "idx23": _wrap16(idx23),
            "M23": M23,
            "wself": wself.reshape(n_win, P).T.copy(),       # [128, n_win]
            "W1r": W1.reshape(4, P, F1).transpose(1, 0, 2).copy(),
            "W2r": W2.reshape(2, P, F2).transpose(1, 0, 2).copy(),
            "W3r": np.ascontiguousarray(W3),
            "W4r": W4.T.reshape(4, P, FO).transpose(1, 0, 2).copy(),
            "b1r": b1.reshape(2, P).T.copy(),
            "b2r": b2.reshape(1, P).T.copy(),
            "b3r": b3.reshape(1, P).T.copy(),
            "b4r": b4.reshape(4, P).T.copy(),
        })

    meta = {
        "npc": npc, "n_win": n_win, "n_pad": n_pad, "ch23": ch23,
        "U_MAX": U_MAX, "s1": s1, "s23": s23, "npc_real": npc_real,
    }
    return cores, meta


# ---------------------------------------------------------------- bass build

DEBUG = False
REPEAT = 1

F32 = mybir.dt.float32
F32R = mybir.dt.float32r
I16 = mybir.dt.int16


def build_bass(meta):
    npc, n_win, n_pad, ch23, U_MAX = (
        meta["npc"], meta["n_win"], meta["n_pad"], meta["ch23"], meta["U_MAX"])
    s1: Sched = meta["s1"]
    s23: Sched = meta["s23"]

    nc = bacc.Bacc("TRN2", target_bir_lowering=False, debug=False,
                   num_devices=NCORES)

    # inputs
    x_tab = nc.dram_tensor("x_tab", [U_MAX, F0], F32R, kind="ExternalInput")
    x_own = nc.dram_tensor("x_own", [npc, F0], F32, kind="ExternalInput")
    idx1 = nc.dram_tensor("idx1", [P, s1.total_slots // 16], I16, kind="ExternalInput")
    M1 = nc.dram_tensor("M1", [P, s1.total_slots], F32R, kind="ExternalInput")
    idx23 = nc.dram_tensor("idx23", [P, s23.total_slots // 16], I16, kind="ExternalInput")
    M23 = nc.dram_tensor("M23", [P, s23.total_slots], F32R, kind="ExternalInput")
    wself = nc.dram_tensor("wself", [P, n_win], F32, kind="ExternalInput")
    W1r = nc.dram_tensor("W1r", [P, 4, F1], F32R, kind="ExternalInput")
    W2r = nc.dram_tensor("W2r", [P, 2, F2], F32R, kind="ExternalInput")
    W3r = nc.dram_tensor("W3r", [P, F2], F32R, kind="ExternalInput")
    W4r = nc.dram_tensor("W4r", [P, 4, FO], F32R, kind="ExternalInput")
    b1r = nc.dram_tensor("b1r", [P, 2], F32, kind="ExternalInput")
    b2r = nc.dram_tensor("b2r", [P, 1], F32, kind="ExternalInput")
    b3r = nc.dram_tensor("b3r", [P, 1], F32, kind="ExternalInput")
    b4r = nc.dram_tensor("b4r", [P, 4], F32, kind="ExternalInput")

    # internal DRAM
    x1T_d = nc.dram_tensor("x1T_d", [P, 2, npc], F32R)
    x2T_d = nc.dram_tensor("x2T_d", [P, npc], F32R)
    g2_own = nc.dram_tensor("g2_own", [npc, F2], F32)
    g3_own = nc.dram_tensor("g3_own", [npc, F3], F32)
    g2_full = nc.dram_tensor("g2_full", [n_pad, F2], F32, addr_space="Shared")
    g3_full = nc.dram_tensor("g3_full", [n_pad, F3], F32, addr_space="Shared")

    # output: feature-major [p, fo, n] == out.T[fo*128+p, n]
    outT = nc.dram_tensor("outT", [P, 4, npc], F32, kind="ExternalOutput")
    if DEBUG:
        dbg_x1T = nc.dram_tensor("dbg_x1T", [P, 2, npc], F32, kind="ExternalOutput")
        dbg_g2own = nc.dram_tensor("dbg_g2own", [npc, F2], F32, kind="ExternalOutput")
        dbg_g2full = nc.dram_tensor("dbg_g2full", [1024, F2], F32, kind="ExternalOutput")
        dbg_agg1 = nc.dram_tensor("dbg_agg1", [P, F0], F32, kind="ExternalOutput")

    rg = [list(range(NCORES))]

    with tile.TileContext(nc) as tc:
        with tc.tile_pool(name="const", bufs=1) as cp, \
             tc.tile_pool(name="sb", bufs=2) as sb, \
             tc.tile_pool(name="sb3", bufs=3) as sb3, \
             tc.tile_pool(name="psA", bufs=3, space="PSUM") as psA, \
             tc.tile_pool(name="psT", bufs=2, space="PSUM") as psT, \
             tc.tile_pool(name="psX", bufs=3, space="PSUM") as psX:

            ident = cp.tile([P, P], F32)
            make_identity(nc, ident[:])

            # resident loads
            idx1_t = cp.tile([P, s1.total_slots // 16], I16)
            nc.sync.dma_start(out=idx1_t[:], in_=idx1[:, :])
            idx23_t = cp.tile([P, s23.total_slots // 16], I16)
            nc.sync.dma_start(out=idx23_t[:], in_=idx23[:, :])
            wself_t = cp.tile([P, n_win], F32)
            nc.sync.dma_start(out=wself_t[:], in_=wself[:, :])
            W1_t = cp.tile([P, 4, F1], F32R)
            nc.sync.dma_start(out=W1_t[:], in_=W1r[:, :, :])
            W2_t = cp.tile([P, 2, F2], F32R)
            nc.sync.dma_start(out=W2_t[:], in_=W2r[:, :, :])
            W3_t = cp.tile([P, F2], F32R)
            nc.sync.dma_start(out=W3_t[:], in_=W3r[:, :])
            W4_t = cp.tile([P, 4, FO], F32R)
            nc.sync.dma_start(out=W4_t[:], in_=W4r[:, :, :])
            b1_t = cp.tile([P, 2], F32)
            nc.sync.dma_start(out=b1_t[:], in_=b1r[:, :])
            b2_t = cp.tile([P, 1], F32)
            nc.sync.dma_start(out=b2_t[:], in_=b2r[:, :])
            b3_t = cp.tile([P, 1], F32)
            nc.sync.dma_start(out=b3_t[:], in_=b3r[:, :])
            b4_t = cp.tile([P, 4], F32)
            nc.sync.dma_start(out=b4_t[:], in_=b4r[:, :])

            def gather_batch(info, sched, idx_t, table_aps, Fdim, tag):
                """Issue dma_gather calls for one batch; returns G tile."""
                nt = info["n_tiles"]
                G = sb.tile([P, nt, Fdim], F32R, tag=tag)
                for (ch, t_off, t_cnt) in info["calls"]:
                    L = t_cnt * P
                    base = info["slot_base"] + t_off * P
                    nc.gpsimd.dma_gather(
                        out_ap=G[:, t_off:t_off + t_cnt, :],
                        in_ap=table_aps[ch],
                        idxs_ap=idx_t[:, base // 16:(base + L) // 16],
                        num_idxs=L,
                        num_idxs_reg=L,
                        elem_size=Fdim,
                    )
                return G

            def agg_windows(info, sched, G, M_d, Fdim, self_rows, nw):
                """Aggregate: per window PSUM agg + self term -> agg_sb [128, nw*Fdim]."""
                nt = info["n_tiles"]
                Mt = sb.tile([P, nt * P], F32R, tag="Mtile")
                nc.sync.dma_start(
                    out=Mt[:],
                    in_=M_d[:, info["slot_base"]:info["slot_base"] + nt * P])
                agg_sb = sb3.tile([P, nw * Fdim], F32, tag=f"aggsb{Fdim}")
                ps_b = None
                for wi, w in enumerate(info["windows"]):
                    tiles = info["win_tiles"][w]
                    if Fdim == F0:
                        ps = psA.tile([P, Fdim], F32, space="PSUM", tag="agg")
                        out_ap = ps[:]
                    else:
                        if ps_b is None:
                            ps_b = psA.tile([P, nw * Fdim], F32, space="PSUM", tag="agg")
                        out_ap = ps_b[:, wi * Fdim:(wi + 1) * Fdim]
                    for j, t in enumerate(tiles):
                        nc.tensor.matmul(
                            out=out_ap,
                            lhsT=Mt[:, t * P:(t + 1) * P],
                            rhs=G[:, t, :],
                            start=(j == 0), stop=(j == len(tiles) - 1),
                        )
                    # self term: agg_sb slice = psum + wself*x_own_window
                    xw = sb.tile([P, Fdim], F32, tag=f"xwin{Fdim}")
                    nc.sync.dma_start(out=xw[:], in_=self_rows(w))
                    tmp = sb.tile([P, Fdim], F32, tag=f"tmp{Fdim}")
                    nc.vector.tensor_scalar_mul(tmp[:], xw[:], wself_t[:, w:w + 1])
                    if tiles:
                        nc.vector.tensor_tensor(
                            out=agg_sb[:, wi * Fdim:(wi + 1) * Fdim],
                            in0=out_ap, in1=tmp[:], op=mybir.AluOpType.add)
                    else:
                        nc.vector.tensor_copy(
                            out=agg_sb[:, wi * Fdim:(wi + 1) * Fdim], in_=tmp[:])
                return agg_sb

            for _rep in range(REPEAT):
                # ---------------- stage A: L1 agg + transform + h2
                for info in s1.batch_info:
                    nw = len(info["windows"])
                    G = gather_batch(info, s1, idx1_t, [x_tab[:, :]], F0, "G1")
                    agg_sb = agg_windows(
                        info, s1, G, M1, F0,
                        lambda w: x_own[w * P:(w + 1) * P, :], nw)
                    # transpose agg -> aggT [128, 4, nw*128] f32r
                    aggT = sb.tile([P, 4, nw * P], F32R, tag="aggT")
                    for wi in range(nw):
                        for kf in range(4):
                            pt = psT.tile([P, P], F32, space="PSUM", tag="tr")
                            nc.tensor.transpose(
                                out=pt[:],
                                in_=agg_sb[:, wi * F0 + kf * P: wi * F0 + (kf + 1) * P],
                                identity=ident[:])
                            nc.vector.tensor_copy(
                                out=aggT[:, kf, wi * P:(wi + 1) * P], in_=pt[:])
                    # x1T = relu(W1.T @ aggT + b1)
                    ncol = nw * P
                    x1T_sb = sb.tile([P, 2, ncol], F32R, tag="x1T")
                    for fo in range(2):
                        px = psX.tile([P, ncol], F32, space="PSUM", tag="xf")
                        for kin in range(4):
                            nc.tensor.matmul(
                                out=px[:],
                                lhsT=W1_t[:, kin, fo * P:(fo + 1) * P],
                                rhs=aggT[:, kin, :],
                                start=(kin == 0), stop=(kin == 3))
                        nc.scalar.activation(
                            out=x1T_sb[:, fo, :], in_=px[:],
                            func=mybir.ActivationFunctionType.Relu,
                            bias=b1_t[:, fo:fo + 1], scale=1.0)
                    c0 = info["windows"][0] * P
                    nc.sync.dma_start(out=x1T_d[:, :, c0:c0 + ncol], in_=x1T_sb[:])
                    if DEBUG:
                        nc.sync.dma_start(
                            out=dbg_x1T[:, :, c0:c0 + ncol], in_=x1T_sb[:].bitcast(F32))
                        if c0 == 0:
                            nc.sync.dma_start(out=dbg_agg1[:, :], in_=agg_sb[:, 0:F0])
                    # h2T = W2.T @ x1T
                    ph = psX.tile([P, ncol], F32, space="PSUM", tag="xf")
                    for kin in range(2):
                        nc.tensor.matmul(
                            out=ph[:], lhsT=W2_t[:, kin, :], rhs=x1T_sb[:, kin, :],
                            start=(kin == 0), stop=(kin == 1))
                    h2T_sb = sb.tile([P, ncol], F32, tag="h2T")
                    nc.vector.tensor_copy(out=h2T_sb[:], in_=ph[:])
                    # transpose h2T -> g2_own rows
                    for wi, w in enumerate(info["windows"]):
                        pt = psT.tile([P, P], F32, space="PSUM", tag="tr")
                        nc.tensor.transpose(
                            out=pt[:], in_=h2T_sb[:, wi * P:(wi + 1) * P],
                            identity=ident[:])
                        hn = sb.tile([P, F2], F32, tag="hn")
                        nc.vector.tensor_copy(out=hn[:], in_=pt[:])
                        nc.sync.dma_start(
                            out=g2_own[w * P:(w + 1) * P, :], in_=hn[:])
                        if DEBUG:
                            nc.sync.dma_start(
                                out=dbg_g2own[w * P:(w + 1) * P, :], in_=hn[:])

                # ---------------- AllGather h2
                nc.gpsimd.collective_compute(
                    "AllGather", mybir.AluOpType.bypass, replica_groups=rg,
                    ins=[g2_own[:, :]], outs=[g2_full[:, :]])

                core_base = None  # own rows live at rank*npc in g*_full; use cc rank trick

                # For self rows in stages B/C we need this core's base offset in
                # g*_full. SPMD program is identical across cores, so read own rows
                # from g*_own instead (same data, core-local).

                def stageBC(sched, idx_t, M_d, g_full, g_own, bias_t, is_final):
                    ch_aps = [g_full[ch * ch23:(ch + 1) * ch23, :].bitcast(F32R)
                              for ch in range(4)]
                    for info in sched.batch_info:
                        nw = len(info["windows"])
                        G = gather_batch(info, sched, idx_t, ch_aps, F2, "G23")
                        agg_sb = agg_windows(
                            info, sched, G, M_d, F2,
                            lambda w: g_own[w * P:(w + 1) * P, :], nw)
                        ncol = nw * P
                        # xT = relu(aggT + b)
                        xT_sb = sb.tile([P, ncol], F32R, tag="xT")
                        for wi in range(nw):
                            pt = psT.tile([P, P], F32, space="PSUM", tag="tr")
                            nc.tensor.transpose(
                                out=pt[:], in_=agg_sb[:, wi * F2:(wi + 1) * F2],
                                identity=ident[:])
                            nc.scalar.activation(
                                out=xT_sb[:, wi * P:(wi + 1) * P], in_=pt[:],
                                func=mybir.ActivationFunctionType.Relu,
                                bias=bias_t[:, 0:1], scale=1.0)
                        c0 = info["windows"][0] * P
                        if not is_final:
                            # stage B: save x2T, compute h3T -> g3_own
                            nc.sync.dma_start(
                                out=x2T_d[:, c0:c0 + ncol], in_=xT_sb[:])
                            ph = psX.tile([P, ncol], F32, space="PSUM", tag="xf")
                            nc.tensor.matmul(out=ph[:], lhsT=W3_t[:], rhs=xT_sb[:],
                                             start=True, stop=True)
                            hT_sb = sb.tile([P, ncol], F32, tag="h2T")
                            nc.vector.tensor_copy(out=hT_sb[:], in_=ph[:])
                            for wi, w in enumerate(info["windows"]):
                                pt = psT.tile([P, P], F32, space="PSUM", tag="tr")
                                nc.tensor.transpose(
                                    out=pt[:], in_=hT_sb[:, wi * P:(wi + 1) * P],
                                    identity=ident[:])
                                hn = sb.tile([P, F3], F32, tag="hn")
                                nc.vector.tensor_copy(out=hn[:], in_=pt[:])
                                nc.sync.dma_start(
                                    out=g3_own[w * P:(w + 1) * P, :], in_=hn[:])
                        else:
                            # stage C: out4T = W4 @ [x1;x2;x3]T + b4
                            x1_t = sb.tile([P, 2, ncol], F32R, tag="x1Tin")
                            nc.sync.dma_start(
                                out=x1_t[:], in_=x1T_d[:, :, c0:c0 + ncol])
                            x2_t = sb.tile([P, ncol], F32R, tag="x2Tin")
                            nc.sync.dma_start(
                                out=x2_t[:], in_=x2T_d[:, c0:c0 + ncol])
                            out_sb = sb.tile([P, 4, ncol], F32, tag="outsb")
                            for fo in range(4):
                                po = psX.tile([P, ncol], F32, space="PSUM", tag="xf")
                                for kin in range(4):
                                    rhs = (x1_t[:, kin, :] if kin < 2 else
                                           x2_t[:] if kin == 2 else xT_sb[:])
                                    nc.tensor.matmul(
                                        out=po[:],
                                        lhsT=W4_t[:, kin, fo * P:(fo + 1) * P],
                                        rhs=rhs, start=(kin == 0), stop=(kin == 3))
                                nc.scalar.activation(
                                    out=out_sb[:, fo, :], in_=po[:],
                                    func=mybir.ActivationFunctionType.Identity,
                                    bias=b4_t[:, fo:fo + 1], scale=1.0)
                            nc.sync.dma_start(
                                out=outT[:, :, c0:c0 + ncol], in_=out_sb[:])

                if DEBUG:
                    for i in range(8):
                        gt = sb.tile([P, F2], F32, tag="dbgt")
                        nc.sync.dma_start(out=gt[:], in_=g2_full[i * P:(i + 1) * P, :])
                        nc.sync.dma_start(out=dbg_g2full[i * P:(i + 1) * P, :], in_=gt[:])

                # ---------------- stage B: L2
                stageBC(s23, idx23_t, M23, g2_full, g2_own, b2_t, is_final=False)

                # ---------------- AllGather h3
                nc.gpsimd.collective_compute(
                    "AllGather", mybir.AluOpType.bypass, replica_groups=rg,
                    ins=[g3_own[:, :]], outs=[g3_full[:, :]])

                # ---------------- stage C: L3 + final
                stageBC(s23, idx23_t, M23, g3_full, g3_own, b3_t, is_final=True)

    nc.compile()
    return nc


# ---------------------------------------------------------------- execution

_EXEC_CACHE = {}


def _make_runner(nc, in_maps):
    """Vendored multi-core bass2jax path with cached jit + device inputs
    (no donation so device buffers are reusable across timed calls)."""
    import jax
    from jax.sharding import Mesh, PartitionSpec
    from jax.experimental.shard_map import shard_map
    from concourse import bass2jax
    from concourse.bass2jax import _bass_exec_p, install_neuronx_cc_hook

    install_neuronx_cc_hook()
    n_cores = len(in_maps)

    partition_name = (nc.partition_id_tensor.name
                      if nc.partition_id_tensor else None)
    in_names, out_names, out_avals = [], [], []
    for alloc in nc.m.functions[0].allocations:
        if not isinstance(alloc, mybir.MemoryLocationSet):
            continue
        name = alloc.memorylocations[0].name
        if alloc.kind == "ExternalInput":
            if name != partition_name:
                in_names.append(name)
        elif alloc.kind == "ExternalOutput":
            out_names.append(name)
            shape = tuple(alloc.tensor_shape)
            dtype = mybir.dt.np(alloc.dtype)
            out_avals.append(jax.core.ShapedArray(shape, dtype))
    n_params = len(in_names)
    all_in_names = list(in_names) + out_names
    if partition_name is not None:
        all_in_names.append(partition_name)

    import jax.numpy as jnp
    from jax.sharding import NamedSharding

    def _body(*args):
        operands = list(args)
        if partition_name is not None:
            operands.append(bass2jax.partition_id_tensor())
        outs = _bass_exec_p.bind(
            *operands,
            out_avals=tuple(out_avals),
            in_names=tuple(all_in_names),
            out_names=tuple(out_names),
            lowering_input_output_aliases=(),
            sim_require_finite=True,
            sim_require_nnan=True,
            nc=nc,
        )
        return tuple(outs)

    devices = jax.devices()[:n_cores]
    mesh = Mesh(np.asarray(devices), ("core",))
    nin = n_params + len(out_names)
    donate = tuple(range(n_params, nin))
    sharded = jax.jit(shard_map(
        _body, mesh=mesh,
        in_specs=(PartitionSpec("core"),) * nin,
        out_specs=(PartitionSpec("core"),) * len(out_names),
        check_rep=False), donate_argnums=donate, keep_unused=True)

    concat_in = [np.concatenate([np.asarray(in_maps[c][nm])
                                 for c in range(n_cores)], axis=0)
                 for nm in in_names]
    dev_args = [jax.device_put(a) for a in concat_in]

    out_shard = NamedSharding(mesh, PartitionSpec("core"))
    zeros_fn = jax.jit(
        lambda: tuple(
            jnp.zeros((n_cores * a.shape[0], *a.shape[1:]), a.dtype)
            for a in out_avals),
        out_shardings=(out_shard,) * len(out_avals))

    def make_zeros():
        zs = zeros_fn()
        jax.block_until_ready(zs)
        return zs

    def exec_with(zs):
        outs = sharded(*dev_args, *zs)
        jax.block_until_ready(outs)
        return outs

    def run():
        outs = exec_with(make_zeros())
        return {nm: np.asarray(outs[i]) for i, nm in enumerate(out_names)}

    run.make_zeros = make_zeros
    run.exec_with = exec_with
    return run, out_avals, out_names


def _assemble(outT_concat, meta):
    npc, npc_real = meta["npc"], meta["npc_real"]
    per_core = outT_concat.reshape(NCORES, P, 4, npc)
    rows = []
    for c in range(NCORES):
        ft = per_core[c].transpose(1, 0, 2).reshape(4 * P, npc)  # [512, npc]
        rows.append(ft.T[:npc_real])
    return np.concatenate(rows, axis=0)


def kernel(x, W1, b1, W2, b2, W3, b3, W4, b4, edge_index, _cache_key=None):
    x = np.asarray(x, np.float32)
    edge_index = np.asarray(edge_index)
    args = [np.asarray(a, np.float32) for a in (W1, b1, W2, b2, W3, b3, W4, b4)]
    npc_real = x.shape[0] // NCORES

    key = _cache_key
    if key is not None and key in _EXEC_CACHE:
        run, meta = _EXEC_CACHE[key]
    else:
        cores, meta = host_prep(x, *args, edge_index, npc_real)
        nc = build_bass(meta)
        run, _, _ = _make_runner(nc, cores)
        if key is not None:
            _EXEC_CACHE[key] = (run, meta)
    out = run()
    return _assemble(out["outT"], meta).astype(np.float32)



# revision 2
# speedup vs baseline: 38.4814x; 38.4814x over previous
"""GCN (3x GCNConv + 1x1 conv) on 8 Trainium2 NeuronCores.

Strategy: node-parallel sharding (12544 padded nodes/core). Symmetric GCN
normalization folds into per-edge weights w_e = dinv[src]*dinv[dst], baked by
the host into sparse "aggregation matrices" M ([128 edge-slots, 128 dst-node]
tiles). On-device aggregation = M.T @ G on the TensorEngine (G = src rows
gathered via dma_gather), accumulated in PSUM; self-loop term is a fused
scale-add from a contiguous window load. Layer 1 aggregates raw x first
(aggregate-then-transform; x replicated to all cores), layers 2/3 are
transform-first with on-device AllGathers of h2/h3. Final 512x512 layer
consumes feature-major x1T/x2T/x3T; output returned feature-major per core and
transposed/assembled on the host.
"""

import math
import sys

import numpy as np

if "/opt/trn_rl_repo" not in sys.path:
    sys.path.insert(0, "/opt/trn_rl_repo")

import concourse.bacc as bacc
import concourse.bass as bass
import concourse.mybir as mybir
import concourse.tile as tile
from concourse.masks import make_identity

P = 128          # partition width / window size
NCORES = 8
F0, F1, F2, F3, FO = 512, 256, 128, 128, 512

# batch tile budgets (slots of 128)
B1_TILES = 8     # L1 gather batch budget ([128, 10, 512] f32 = 2.6MB)
B23_TILES = 16   # L2/3 gather batch budget ([128, 18, 128] f32 = 1.2MB)


# ---------------------------------------------------------------- host prep

def _pack_batches(tiles_per_window, budget):
    """Greedy pack windows into batches with sum(tiles) <= budget."""
    batches = []
    cur, cur_t = [], 0
    for w, t in enumerate(tiles_per_window):
        if cur and cur_t + t > budget:
            batches.append(cur)
            cur, cur_t = [], 0
        cur.append(w)
        cur_t += t
    if cur:
        batches.append(cur)
    return batches


class Sched:
    """Static (core-independent) slot schedule for one gather stream.

    Layout: batches of windows; within a batch, tiles are laid chunk-major:
    for ch in chunks: for w in batch: T[w,ch] tiles. Every (w,ch) run is a
    whole number of 128-slot tiles, so tiles never straddle windows.
    """

    def __init__(self, T_wc, budget):
        # T_wc: [n_windows, n_chunks] tile counts
        self.T_wc = T_wc
        self.n_windows, self.n_chunks = T_wc.shape
        self.batches = _pack_batches(T_wc.sum(axis=1), budget)
        self.batch_info = []   # per batch dicts
        # global tile base of (w, ch)
        self.tile_base = np.zeros((self.n_windows, self.n_chunks), np.int64)
        gt = 0
        for bw in self.batches:
            info = {"windows": bw, "slot_base": gt * P, "calls": [],
                    "win_tiles": {w: [] for w in bw}}
            bt = 0  # batch-local tile idx
            for ch in range(self.n_chunks):
                call_tiles = 0
                call_off = bt
                for w in bw:
                    t = int(T_wc[w, ch])
                    self.tile_base[w, ch] = gt + bt
                    for k in range(t):
                        info["win_tiles"][w].append(bt + k)
                    bt += t
                    call_tiles += t
                if call_tiles > 0:
                    info["calls"].append((ch, call_off, call_tiles))
            info["n_tiles"] = bt
            self.batch_info.append(info)
            gt += bt
        self.total_tiles = gt
        self.total_slots = gt * P


def _group_ranks(keys):
    """For sorted keys, rank of each element within its group."""
    n = len(keys)
    if n == 0:
        return np.zeros(0, np.int64)
    starts = np.r_[0, np.flatnonzero(np.diff(keys)) + 1]
    lens = np.diff(np.r_[starts, n])
    return np.arange(n) - np.repeat(starts, lens)


def _fill_stream(sched, w_e, ch_e, col_e, wt_e, rel_e):
    """Given a core's edges (window, chunk, col, weight, rel table idx),
    produce idx_flat [total_slots] int16 and M [128, total_slots] f32."""
    S = sched.total_slots
    idx_flat = np.zeros(S, np.int16)
    M = np.zeros((P, S), np.float32)
    if len(w_e):
        key = w_e.astype(np.int64) * sched.n_chunks + ch_e
        order = np.argsort(key, kind="stable")
        ks = key[order]
        ranks = _group_ranks(ks)
        tb = sched.tile_base[w_e[order], ch_e[order]]
        slot = (tb + ranks // P) * P + ranks % P
        idx_flat[slot] = rel_e[order].astype(np.int16)
        M[slot % P, (slot // P) * P + col_e[order]] = wt_e[order]
    return idx_flat, M


def _wrap16(idx_flat):
    """[S] -> [128, S//16] int16 (wrapped in 16 partitions, replicated x8)."""
    S = len(idx_flat)
    return np.tile(idx_flat.reshape(S // 16, 16).T, (8, 1)).copy()


def host_prep(x, W1, b1, W2, b2, W3, b3, W4, b4, edge_index, npc_real):
    N = x.shape[0]
    ncores = NCORES
    npc = ((npc_real + P - 1) // P) * P      # padded nodes per core
    n_win = npc // P
    n_pad = npc * ncores
    ch23 = n_pad // 4                        # L2/3 chunk size
    assert ch23 <= 32768 and n_pad % 4 == 0

    src = np.asarray(edge_index[0], np.int64)
    dst = np.asarray(edge_index[1], np.int64)
    deg = np.bincount(dst, minlength=N).astype(np.float64) + 1.0
    dinv = (1.0 / np.sqrt(deg)).astype(np.float32)

    def pad_id(v):
        return (v // npc_real) * npc + (v % npc_real)

    src_p = pad_id(src)
    dst_p = pad_id(dst)
    w_edge = (dinv[src] * dinv[dst]).astype(np.float32)

    x_pad = np.zeros((n_pad, F0), np.float32)
    for c in range(ncores):
        x_pad[c * npc:c * npc + npc_real] = x[c * npc_real:(c + 1) * npc_real]

    # per-core edge partitions
    core_of = dst // npc_real
    per_core = []
    for c in range(ncores):
        m = core_of == c
        per_core.append({
            "src_p": src_p[m],
            "dstrel": dst_p[m] - c * npc,
            "w": w_edge[m],
        })

    # ---- L1 compact tables
    uniq_list, inv_list = [], []
    for c in range(ncores):
        u, inv = np.unique(per_core[c]["src_p"], return_inverse=True)
        uniq_list.append(u)
        inv_list.append(inv)
    U_MAX = max(1, max(len(u) for u in uniq_list))
    assert U_MAX <= 32768, f"compact table too big: {U_MAX}"

    # ---- schedules (global max over cores)
    cnt1 = np.zeros((ncores, n_win), np.int64)
    cnt23 = np.zeros((ncores, n_win, 4), np.int64)
    for c in range(ncores):
        w_e = per_core[c]["dstrel"] // P
        ch_e = per_core[c]["src_p"] // ch23
        np.add.at(cnt1, (c, w_e), 1)
        np.add.at(cnt23, (c, w_e, ch_e), 1)
    T1 = np.ceil(cnt1.max(axis=0) / P).astype(np.int64)[:, None]   # [n_win,1]
    T23 = np.ceil(cnt23.max(axis=0) / P).astype(np.int64)         # [n_win,4]
    s1 = Sched(T1, B1_TILES)
    s23 = Sched(T23, B23_TILES)

    # ---- per-core arrays
    cores = []
    for c in range(ncores):
        pc = per_core[c]
        w_e = (pc["dstrel"] // P).astype(np.int64)
        col_e = (pc["dstrel"] % P).astype(np.int64)

        idx1, M1 = _fill_stream(
            s1, w_e, np.zeros_like(w_e), col_e, pc["w"], inv_list[c])
        ch_e = (pc["src_p"] // ch23).astype(np.int64)
        rel_e = (pc["src_p"] % ch23).astype(np.int64)
        idx23, M23 = _fill_stream(s23, w_e, ch_e, col_e, pc["w"], rel_e)

        tab = np.zeros((U_MAX, F0), np.float32)
        tab[:len(uniq_list[c])] = x_pad[uniq_list[c]]

        wself = np.zeros(npc, np.float32)
        wself[:npc_real] = dinv[c * npc_real:(c + 1) * npc_real] ** 2
        cores.append({
            "x_tab": tab,
            "x_own": x_pad[c * npc:(c + 1) * npc],
            "idx1": _wrap16(idx1),
            "M1": M1,
            "idx23": _wrap16(idx23),
            "M23": M23,
            "wself": wself.reshape(n_win, P).T.copy(),       # [128, n_win]
            "W1r": W1.reshape(4, P, F1).transpose(1, 0, 2).copy(),
            "W2r": W2.reshape(2, P, F2).transpose(1, 0, 2).copy(),
            "W3r": np.ascontiguousarray(W3),
            "W4r": W4.T.reshape(4, P, FO).transpose(1, 0, 2).copy(),
            "b1r": b1.reshape(2, P).T.copy(),
            "b2r": b2.reshape(1, P).T.copy(),
            "b3r": b3.reshape(1, P).T.copy(),
            "b4r": b4.reshape(4, P).T.copy(),
        })

    meta = {
        "npc": npc, "n_win": n_win, "n_pad": n_pad, "ch23": ch23,
        "U_MAX": U_MAX, "s1": s1, "s23": s23, "npc_real": npc_real,
    }
    return cores, meta


# ---------------------------------------------------------------- bass build

DEBUG = False
REPEAT = 1

F32 = mybir.dt.float32
F32R = mybir.dt.float32r
I16 = mybir.dt.int16


def build_bass(meta):
    npc, n_win, n_pad, ch23, U_MAX = (
        meta["npc"], meta["n_win"], meta["n_pad"], meta["ch23"], meta["U_MAX"])
    s1: Sched = meta["s1"]
    s23: Sched = meta["s23"]

    nc = bacc.Bacc("TRN2", target_bir_lowering=False, debug=False,
                   num_devices=NCORES)

    # inputs
    x_tab = nc.dram_tensor("x_tab", [U_MAX, F0], F32R, kind="ExternalInput")
    x_own = nc.dram_tensor("x_own", [npc, F0], F32, kind="ExternalInput")
    idx1 = nc.dram_tensor("idx1", [P, s1.total_slots // 16], I16, kind="ExternalInput")
    M1 = nc.dram_tensor("M1", [P, s1.total_slots], F32R, kind="ExternalInput")
    idx23 = nc.dram_tensor("idx23", [P, s23.total_slots // 16], I16, kind="ExternalInput")
    M23 = nc.dram_tensor("M23", [P, s23.total_slots], F32R, kind="ExternalInput")
    wself = nc.dram_tensor("wself", [P, n_win], F32, kind="ExternalInput")
    W1r = nc.dram_tensor("W1r", [P, 4, F1], F32R, kind="ExternalInput")
    W2r = nc.dram_tensor("W2r", [P, 2, F2], F32R, kind="ExternalInput")
    W3r = nc.dram_tensor("W3r", [P, F2], F32R, kind="ExternalInput")
    W4r = nc.dram_tensor("W4r", [P, 4, FO], F32R, kind="ExternalInput")
    b1r = nc.dram_tensor("b1r", [P, 2], F32, kind="ExternalInput")
    b2r = nc.dram_tensor("b2r", [P, 1], F32, kind="ExternalInput")
    b3r = nc.dram_tensor("b3r", [P, 1], F32, kind="ExternalInput")
    b4r = nc.dram_tensor("b4r", [P, 4], F32, kind="ExternalInput")

    # internal DRAM
    x1T_d = nc.dram_tensor("x1T_d", [P, 2, npc], F32R)
    x2T_d = nc.dram_tensor("x2T_d", [P, npc], F32R)
    g2_own = nc.dram_tensor("g2_own", [npc, F2], F32)
    g3_own = nc.dram_tensor("g3_own", [npc, F3], F32)
    g2_full = nc.dram_tensor("g2_full", [n_pad, F2], F32, addr_space="Shared")
    g3_full = nc.dram_tensor("g3_full", [n_pad, F3], F32, addr_space="Shared")

    # output: feature-major [p, fo, n] == out.T[fo*128+p, n]
    outT = nc.dram_tensor("outT", [P, 4, npc], F32, kind="ExternalOutput")
    if DEBUG:
        dbg_x1T = nc.dram_tensor("dbg_x1T", [P, 2, npc], F32, kind="ExternalOutput")
        dbg_g2own = nc.dram_tensor("dbg_g2own", [npc, F2], F32, kind="ExternalOutput")
        dbg_g2full = nc.dram_tensor("dbg_g2full", [1024, F2], F32, kind="ExternalOutput")
        dbg_agg1 = nc.dram_tensor("dbg_agg1", [P, F0], F32, kind="ExternalOutput")

    rg = [list(range(NCORES))]

    with tile.TileContext(nc) as tc:
        with tc.tile_pool(name="const", bufs=1) as cp, \
             tc.tile_pool(name="sb", bufs=2) as sb, \
             tc.tile_pool(name="sb3", bufs=3) as sb3, \
             tc.tile_pool(name="psA", bufs=3, space="PSUM") as psA, \
             tc.tile_pool(name="psT", bufs=2, space="PSUM") as psT, \
             tc.tile_pool(name="psX", bufs=3, space="PSUM") as psX:

            ident = cp.tile([P, P], F32)
            make_identity(nc, ident[:])

            # resident loads
            idx1_t = cp.tile([P, s1.total_slots // 16], I16)
            nc.sync.dma_start(out=idx1_t[:], in_=idx1[:, :])
            idx23_t = cp.tile([P, s23.total_slots // 16], I16)
            nc.sync.dma_start(out=idx23_t[:], in_=idx23[:, :])
            wself_t = cp.tile([P, n_win], F32)
            nc.sync.dma_start(out=wself_t[:], in_=wself[:, :])
            W1_t = cp.tile([P, 4, F1], F32R)
            nc.sync.dma_start(out=W1_t[:], in_=W1r[:, :, :])
            W2_t = cp.tile([P, 2, F2], F32R)
            nc.sync.dma_start(out=W2_t[:], in_=W2r[:, :, :])
            W3_t = cp.tile([P, F2], F32R)
            nc.sync.dma_start(out=W3_t[:], in_=W3r[:, :])
            W4_t = cp.tile([P, 4, FO], F32R)
            nc.sync.dma_start(out=W4_t[:], in_=W4r[:, :, :])
            b1_t = cp.tile([P, 2], F32)
            nc.sync.dma_start(out=b1_t[:], in_=b1r[:, :])
            b2_t = cp.tile([P, 1], F32)
            nc.sync.dma_start(out=b2_t[:], in_=b2r[:, :])
            b3_t = cp.tile([P, 1], F32)
            nc.sync.dma_start(out=b3_t[:], in_=b3r[:, :])
            b4_t = cp.tile([P, 4], F32)
            nc.sync.dma_start(out=b4_t[:], in_=b4r[:, :])

            def gather_batch(info, sched, idx_t, table_aps, Fdim, tag):
                """Issue dma_gather calls for one batch; returns G tile."""
                nt = info["n_tiles"]
                G = sb.tile([P, nt, Fdim], F32R, tag=tag)
                for (ch, t_off, t_cnt) in info["calls"]:
                    L = t_cnt * P
                    base = info["slot_base"] + t_off * P
                    nc.gpsimd.dma_gather(
                        out_ap=G[:, t_off:t_off + t_cnt, :],
                        in_ap=table_aps[ch],
                        idxs_ap=idx_t[:, base // 16:(base + L) // 16],
                        num_idxs=L,
                        num_idxs_reg=L,
                        elem_size=Fdim,
                    )
                return G

            def agg_windows(info, sched, G, M_d, Fdim, self_rows, nw):
                """Aggregate: per window PSUM agg + self term -> agg_sb [128, nw*Fdim]."""
                nt = info["n_tiles"]
                Mt = sb.tile([P, nt * P], F32R, tag="Mtile")
                nc.sync.dma_start(
                    out=Mt[:],
                    in_=M_d[:, info["slot_base"]:info["slot_base"] + nt * P])
                agg_sb = sb3.tile([P, nw * Fdim], F32, tag=f"aggsb{Fdim}")
                ps_b = None
                for wi, w in enumerate(info["windows"]):
                    tiles = info["win_tiles"][w]
                    if Fdim == F0:
                        ps = psA.tile([P, Fdim], F32, space="PSUM", tag="agg")
                        out_ap = ps[:]
                    else:
                        if ps_b is None:
                            ps_b = psA.tile([P, nw * Fdim], F32, space="PSUM", tag="agg")
                        out_ap = ps_b[:, wi * Fdim:(wi + 1) * Fdim]
                    for j, t in enumerate(tiles):
                        nc.tensor.matmul(
                            out=out_ap,
                            lhsT=Mt[:, t * P:(t + 1) * P],
                            rhs=G[:, t, :],
                            start=(j == 0), stop=(j == len(tiles) - 1),
                        )
                    # self term: agg_sb slice = psum + wself*x_own_window
                    xw = sb.tile([P, Fdim], F32, tag=f"xwin{Fdim}")
                    nc.sync.dma_start(out=xw[:], in_=self_rows(w))
                    tmp = sb.tile([P, Fdim], F32, tag=f"tmp{Fdim}")
                    nc.vector.tensor_scalar_mul(tmp[:], xw[:], wself_t[:, w:w + 1])
                    if tiles:
                        nc.vector.tensor_tensor(
                            out=agg_sb[:, wi * Fdim:(wi + 1) * Fdim],
                            in0=out_ap, in1=tmp[:], op=mybir.AluOpType.add)
                    else:
                        nc.vector.tensor_copy(
                            out=agg_sb[:, wi * Fdim:(wi + 1) * Fdim], in_=tmp[:])
                return agg_sb

            for _rep in range(REPEAT):
                # ---------------- stage A: L1 agg + transform + h2
                for info in s1.batch_info:
                    nw = len(info["windows"])
                    G = gather_batch(info, s1, idx1_t, [x_tab[:, :]], F0, "G1")
                    agg_sb = agg_windows(
                        info, s1, G, M1, F0,
                        lambda w: x_own[w * P:(w + 1) * P, :], nw)
                    # transpose agg -> aggT [128, 4, nw*128] f32r
                    aggT = sb.tile([P, 4, nw * P], F32R, tag="aggT")
                    for wi in range(nw):
                        for kf in range(4):
                            pt = psT.tile([P, P], F32, space="PSUM", tag="tr")
                            nc.tensor.transpose(
                                out=pt[:],
                                in_=agg_sb[:, wi * F0 + kf * P: wi * F0 + (kf + 1) * P],
                                identity=ident[:])
                            nc.vector.tensor_copy(
                                out=aggT[:, kf, wi * P:(wi + 1) * P], in_=pt[:])
                    # x1T = relu(W1.T @ aggT + b1)
                    ncol = nw * P
                    x1T_sb = sb.tile([P, 2, ncol], F32R, tag="x1T")
                    for fo in range(2):
                        px = psX.tile([P, ncol], F32, space="PSUM", tag="xf")
                        for kin in range(4):
                            nc.tensor.matmul(
                                out=px[:],
                                lhsT=W1_t[:, kin, fo * P:(fo + 1) * P],
                                rhs=aggT[:, kin, :],
                                start=(kin == 0), stop=(kin == 3))
                        nc.scalar.activation(
                            out=x1T_sb[:, fo, :], in_=px[:],
                            func=mybir.ActivationFunctionType.Relu,
                            bias=b1_t[:, fo:fo + 1], scale=1.0)
                    c0 = info["windows"][0] * P
                    nc.sync.dma_start(out=x1T_d[:, :, c0:c0 + ncol], in_=x1T_sb[:])
                    if DEBUG:
                        nc.sync.dma_start(
                            out=dbg_x1T[:, :, c0:c0 + ncol], in_=x1T_sb[:].bitcast(F32))
                        if c0 == 0:
                            nc.sync.dma_start(out=dbg_agg1[:, :], in_=agg_sb[:, 0:F0])
                    # h2T = W2.T @ x1T
                    ph = psX.tile([P, ncol], F32, space="PSUM", tag="xf")
                    for kin in range(2):
                        nc.tensor.matmul(
                            out=ph[:], lhsT=W2_t[:, kin, :], rhs=x1T_sb[:, kin, :],
                            start=(kin == 0), stop=(kin == 1))
                    h2T_sb = sb.tile([P, ncol], F32, tag="h2T")
                    nc.vector.tensor_copy(out=h2T_sb[:], in_=ph[:])
                    # transpose h2T -> g2_own rows
                    for wi, w in enumerate(info["windows"]):
                        pt = psT.tile([P, P], F32, space="PSUM", tag="tr")
                        nc.tensor.transpose(
                            out=pt[:], in_=h2T_sb[:, wi * P:(wi + 1) * P],
                            identity=ident[:])
                        hn = sb.tile([P, F2], F32, tag="hn")
                        nc.vector.tensor_copy(out=hn[:], in_=pt[:])
                        nc.sync.dma_start(
                            out=g2_own[w * P:(w + 1) * P, :], in_=hn[:])
                        if DEBUG:
                            nc.sync.dma_start(
                                out=dbg_g2own[w * P:(w + 1) * P, :], in_=hn[:])

                # ---------------- AllGather h2
                nc.gpsimd.collective_compute(
                    "AllGather", mybir.AluOpType.bypass, replica_groups=rg,
                    ins=[g2_own[:, :]], outs=[g2_full[:, :]])

                core_base = None  # own rows live at rank*npc in g*_full; use cc rank trick

                # For self rows in stages B/C we need this core's base offset in
                # g*_full. SPMD program is identical across cores, so read own rows
                # from g*_own instead (same data, core-local).

                def stageBC(sched, idx_t, M_d, g_full, g_own, bias_t, is_final):
                    ch_aps = [g_full[ch * ch23:(ch + 1) * ch23, :].bitcast(F32R)
                              for ch in range(4)]
                    for info in sched.batch_info:
                        nw = len(info["windows"])
                        G = gather_batch(info, sched, idx_t, ch_aps, F2, "G23")
                        agg_sb = agg_windows(
                            info, sched, G, M_d, F2,
                            lambda w: g_own[w * P:(w + 1) * P, :], nw)
                        ncol = nw * P
                        # xT = relu(aggT + b)
                        xT_sb = sb.tile([P, ncol], F32R, tag="xT")
                        for wi in range(nw):
                            pt = psT.tile([P, P], F32, space="PSUM", tag="tr")
                            nc.tensor.transpose(
                                out=pt[:], in_=agg_sb[:, wi * F2:(wi + 1) * F2],
                                identity=ident[:])
                            nc.scalar.activation(
                                out=xT_sb[:, wi * P:(wi + 1) * P], in_=pt[:],
                                func=mybir.ActivationFunctionType.Relu,
                                bias=bias_t[:, 0:1], scale=1.0)
                        c0 = info["windows"][0] * P
                        if not is_final:
                            # stage B: save x2T, compute h3T -> g3_own
                            nc.sync.dma_start(
                                out=x2T_d[:, c0:c0 + ncol], in_=xT_sb[:])
                            ph = psX.tile([P, ncol], F32, space="PSUM", tag="xf")
                            nc.tensor.matmul(out=ph[:], lhsT=W3_t[:], rhs=xT_sb[:],
                                             start=True, stop=True)
                            hT_sb = sb.tile([P, ncol], F32, tag="h2T")
                            nc.vector.tensor_copy(out=hT_sb[:], in_=ph[:])
                            for wi, w in enumerate(info["windows"]):
                                pt = psT.tile([P, P], F32, space="PSUM", tag="tr")
                                nc.tensor.transpose(
                                    out=pt[:], in_=hT_sb[:, wi * P:(wi + 1) * P],
                                    identity=ident[:])
                                hn = sb.tile([P, F3], F32, tag="hn")
                                nc.vector.tensor_copy(out=hn[:], in_=pt[:])
                                nc.sync.dma_start(
                                    out=g3_own[w * P:(w + 1) * P, :], in_=hn[:])
                        else:
                            # stage C: out4T = W4 @ [x1;x2;x3]T + b4
                            x1_t = sb.tile([P, 2, ncol], F32R, tag="x1Tin")
                            nc.sync.dma_start(
                                out=x1_t[:], in_=x1T_d[:, :, c0:c0 + ncol])
                            x2_t = sb.tile([P, ncol], F32R, tag="x2Tin")
                            nc.sync.dma_start(
                                out=x2_t[:], in_=x2T_d[:, c0:c0 + ncol])
                            out_sb = sb.tile([P, 4, ncol], F32, tag="outsb")
                            for fo in range(4):
                                po = psX.tile([P, ncol], F32, space="PSUM", tag="xf")
                                for kin in range(4):
                                    rhs = (x1_t[:, kin, :] if kin < 2 else
                                           x2_t[:] if kin == 2 else xT_sb[:])
                                    nc.tensor.matmul(
                                        out=po[:],
                                        lhsT=W4_t[:, kin, fo * P:(fo + 1) * P],
                                        rhs=rhs, start=(kin == 0), stop=(kin == 3))
                                nc.scalar.activation(
                                    out=out_sb[:, fo, :], in_=po[:],
                                    func=mybir.ActivationFunctionType.Identity,
                                    bias=b4_t[:, fo:fo + 1], scale=1.0)
                            nc.sync.dma_start(
                                out=outT[:, :, c0:c0 + ncol], in_=out_sb[:])

                if DEBUG:
                    for i in range(8):
                        gt = sb.tile([P, F2], F32, tag="dbgt")
                        nc.sync.dma_start(out=gt[:], in_=g2_full[i * P:(i + 1) * P, :])
                        nc.sync.dma_start(out=dbg_g2full[i * P:(i + 1) * P, :], in_=gt[:])

                # ---------------- stage B: L2
                stageBC(s23, idx23_t, M23, g2_full, g2_own, b2_t, is_final=False)

                # ---------------- AllGather h3
                nc.gpsimd.collective_compute(
                    "AllGather", mybir.AluOpType.bypass, replica_groups=rg,
                    ins=[g3_own[:, :]], outs=[g3_full[:, :]])

                # ---------------- stage C: L3 + final
                stageBC(s23, idx23_t, M23, g3_full, g3_own, b3_t, is_final=True)

    nc.compile()
    return nc


# ---------------------------------------------------------------- execution

_EXEC_CACHE = {}


def _make_runner(nc, in_maps):
    """Vendored multi-core bass2jax path with cached jit + device inputs
    (no donation so device buffers are reusable across timed calls)."""
    import jax
    from jax.sharding import Mesh, PartitionSpec
    from jax.experimental.shard_map import shard_map
    from concourse import bass2jax
    from concourse.bass2jax import _bass_exec_p, install_neuronx_cc_hook

    install_neuronx_cc_hook()
    n_cores = len(in_maps)

    partition_name = (nc.partition_id_tensor.name
                      if nc.partition_id_tensor else None)
    in_names, out_names, out_avals = [], [], []
    for alloc in nc.m.functions[0].allocations:
        if not isinstance(alloc, mybir.MemoryLocationSet):
            continue
        name = alloc.memorylocations[0].name
        if alloc.kind == "ExternalInput":
            if name != partition_name:
                in_names.append(name)
        elif alloc.kind == "ExternalOutput":
            out_names.append(name)
            shape = tuple(alloc.tensor_shape)
            dtype = mybir.dt.np(alloc.dtype)
            out_avals.append(jax.core.ShapedArray(shape, dtype))
    n_params = len(in_names)
    all_in_names = list(in_names) + out_names
    if partition_name is not None:
        all_in_names.append(partition_name)

    import jax.numpy as jnp
    from jax.sharding import NamedSharding

    def _body(*args):
        operands = list(args)
        if partition_name is not None:
            operands.append(bass2jax.partition_id_tensor())
        outs = _bass_exec_p.bind(
            *operands,
            out_avals=tuple(out_avals),
            in_names=tuple(all_in_names),
            out_names=tuple(out_names),
            lowering_input_output_aliases=(),
            sim_require_finite=True,
            sim_require_nnan=True,
            nc=nc,
        )
        return tuple(outs)

    devices = jax.devices()[:n_cores]
    mesh = Mesh(np.asarray(devices), ("core",))
    nin = n_params + len(out_names)
    donate = tuple(range(n_params, nin))
    sharded = jax.jit(shard_map(
        _body, mesh=mesh,
        in_specs=(PartitionSpec("core"),) * nin,
        out_specs=(PartitionSpec("core"),) * len(out_names),
        check_rep=False), donate_argnums=donate, keep_unused=True)

    concat_in = [np.concatenate([np.asarray(in_maps[c][nm])
                                 for c in range(n_cores)], axis=0)
                 for nm in in_names]
    in_shard = NamedSharding(mesh, PartitionSpec("core"))
    dev_args = [jax.device_put(a, in_shard) for a in concat_in]

    out_shard = NamedSharding(mesh, PartitionSpec("core"))
    zeros_fn = jax.jit(
        lambda: tuple(
            jnp.zeros((n_cores * a.shape[0], *a.shape[1:]), a.dtype)
            for a in out_avals),
        out_shardings=(out_shard,) * len(out_avals))

    def make_zeros():
        zs = zeros_fn()
        jax.block_until_ready(zs)
        return zs

    def exec_with(zs):
        outs = sharded(*dev_args, *zs)
        jax.block_until_ready(outs)
        return outs

    def run():
        outs = exec_with(make_zeros())
        return {nm: np.asarray(outs[i]) for i, nm in enumerate(out_names)}

    run.make_zeros = make_zeros
    run.exec_with = exec_with
    return run, out_avals, out_names


def _assemble(outT_concat, meta):
    npc, npc_real = meta["npc"], meta["npc_real"]
    per_core = outT_concat.reshape(NCORES, P, 4, npc)
    rows = []
    for c in range(NCORES):
        ft = per_core[c].transpose(1, 0, 2).reshape(4 * P, npc)  # [512, npc]
        rows.append(ft.T[:npc_real])
    return np.concatenate(rows, axis=0)


def kernel(x, W1, b1, W2, b2, W3, b3, W4, b4, edge_index, _cache_key=None):
    x = np.asarray(x, np.float32)
    edge_index = np.asarray(edge_index)
    args = [np.asarray(a, np.float32) for a in (W1, b1, W2, b2, W3, b3, W4, b4)]
    npc_real = x.shape[0] // NCORES

    key = _cache_key
    if key is not None and key in _EXEC_CACHE:
        run, meta = _EXEC_CACHE[key]
    else:
        cores, meta = host_prep(x, *args, edge_index, npc_real)
        nc = build_bass(meta)
        run, _, _ = _make_runner(nc, cores)
        if key is not None:
            _EXEC_CACHE[key] = (run, meta)
    out = run()
    return _assemble(out["outT"], meta).astype(np.float32)

